# revision 21
# baseline (speedup 1.0000x reference)
"""Multi-branch shared-KV cross-attention for Trainium2, batch-parallel on 8 cores.

Math (per batch b, per branch i with C in {64,128,256,512}):
  K   = emb_all @ Wk[h].T                    [N=1024, KV=960]  per head
  V   = emb_all @ Wv[h].T
  Q   = emb_i  @ (Wq[h]/sqrt(960)).T         [N, C]
  S   = Q.T @ K                              [C, KV]   (1/sqrt(960) folded in Wq)
  P   = softmax_j(rsq * S - rsq*mu),  rsq = 1/sqrt(var(S)+eps)  plane stats
  ctx = P @ V.T                              [C, N]
  out = sum_h (ctx_h.T @ (Wo.T/4))           [N, C]    (mean over 4 heads folded)

Each of the 8 NeuronCores processes one batch element; no collectives.
All heavy matmuls run in float32r (TF32-like, full PE rate at free-dim>=256,
~1.5e-4 l2 error/matmul); statistics, softmax and accumulation in fp32.
"""
import math
import sys

sys.path.insert(0, "/opt/trn_rl_repo")

import numpy as np

import concourse.bass as bass
import concourse.tile as tile
from concourse import bacc, mybir
from concourse.bass_utils import run_bass_kernel_spmd
from concourse.masks import make_identity

F32 = mybir.dt.float32
F32R = mybir.dt.float32r
AF = mybir.ActivationFunctionType
ALU = mybir.AluOpType
ts = bass.ts

B, N, H, KV = 8, 1024, 4, 960
CS = [64, 128, 256, 512]
EPS = 1e-5
NCORES = 8
KT, KP = 8, 120  # contraction tiles over KV: 960 = 8*120
NT = 8           # n tiles: 1024 = 8*128
JT, JP = 8, 120  # j tiles over KV for ctx: 960 = 8*120

TRACE = False  # test.py sets kernel.TRACE = True to profile


def _nct(c):
    return (c + 127) // 128


def _pin_act_tables():
    """Keep Exp/Ln/Square/Copy only in natural_log_exp_and_others so the
    table-load pass never alternates sets (each switch costs ~1.3us on the
    softmax critical path)."""
    from concourse import hw_specs

    if getattr(hw_specs, "_act_tables_pinned", False):
        return
    orig = hw_specs.get_activation_tables

    def patched(arch):
        t = orig(arch)
        mine = {AF.Exp, AF.Ln, AF.Square, AF.Copy, AF.Identity, AF.MemsetZero}
        for name in t:
            if name != "natural_log_exp_and_others":
                t[name] = t[name] - mine
        return t

    hw_specs.get_activation_tables = patched
    bacc.get_activation_tables = patched
    hw_specs._act_tables_pinned = True


def build():
    _pin_act_tables()
    nc = bacc.Bacc("TRN2", target_bir_lowering=False, debug=False, num_devices=NCORES)

    embt_d = [
        nc.dram_tensor(f"embt{i + 1}", [c, N], F32R, kind="ExternalInput").ap()
        for i, c in enumerate(CS)
    ]
    at_d = nc.dram_tensor("at", [KT, KP, N], F32R, kind="ExternalInput").ap()
    wkt_d = nc.dram_tensor("wkt", [H, KT, KP, KV], F32R, kind="ExternalInput").ap()
    wvt_d = nc.dram_tensor("wvt", [H, KT, KP, KV], F32R, kind="ExternalInput").ap()
    wqt_d = [
        nc.dram_tensor(f"wqt{i + 1}", [H, c, c], F32R, kind="ExternalInput").ap()
        for i, c in enumerate(CS)
    ]
    wot_d = [
        nc.dram_tensor(f"wot{i + 1}", [c, c], F32R, kind="ExternalInput").ap()
        for i, c in enumerate(CS)
    ]
    o_d = [
        nc.dram_tensor(f"o{i + 1}", [c, N], F32, kind="ExternalOutput").ap()
        for i, c in enumerate(CS)
    ]

    with tile.TileContext(nc) as tc:
        with (
            tc.tile_pool(name="big", bufs=9) as pbig,
            tc.tile_pool(name="w", bufs=2) as pw,
            tc.tile_pool(name="k", bufs=8) as pk,
            tc.tile_pool(name="v", bufs=8) as pv,
            tc.tile_pool(name="q", bufs=8) as pq,
            tc.tile_pool(name="wq", bufs=4) as pwq,
            tc.tile_pool(name="pt", bufs=1) as ppt,
            tc.tile_pool(name="ctx", bufs=4) as pctx,
            tc.tile_pool(name="wo", bufs=4) as pwo,
            tc.tile_pool(name="osb", bufs=2) as posb,
            tc.tile_pool(name="sm", bufs=4) as psm,
            tc.tile_pool(name="cst", bufs=1) as pcst,
            tc.tile_pool(name="mm", bufs=6, space="PSUM") as pmm,
            tc.tile_pool(name="tp", bufs=2, space="PSUM") as ptp,
        ):
            # ---- constants
            ident_f = pcst.tile([128, 128], F32, tag="identf")
            make_identity(nc, ident_f[:])
            ident_r = pcst.tile([128, 128], F32R, tag="identr")
            nc.vector.tensor_copy(ident_r[:], ident_f[:])
            inv_col = pcst.tile([128, 4], F32, tag="invc")
            for _br, _c in enumerate(CS):
                nc.vector.memset(inv_col[:, _br : _br + 1], 1.0 / (960.0 * _c))
            ones_row = pcst.tile([1, 128], F32, tag="onesr")
            nc.vector.memset(ones_row[:], 1.0)
            eps_t = pcst.tile([1, 1], F32, tag="eps")
            nc.vector.memset(eps_t[:], EPS)

            for h in range(H):
                # ---- first Wk chunk, then AT tiles (PE can start sooner)
                wk0 = pw.tile([KP, KT, 480], F32R, tag="w")
                nc.sync.dma_start(
                    wk0[:], wkt_d[h, :, :, ts(0, 480)].rearrange("a b c -> b a c")
                )
                at_t = []
                for kt in range(KT):
                    a = pbig.tile([KP, N], F32R, tag="big")
                    nc.sync.dma_start(a[:], at_d[kt])
                    at_t.append(a)

                # prefetch the first branch's Q inputs (c=64 only: one spare
                # big slot); its Q/S matmuls then fill the K->V DMA gap
                br_order = (3, 2, 1, 0) if h == H - 1 else (0, 1, 2, 3)
                pre_br = br_order[0] if _nct(CS[br_order[0]]) == 1 else None
                pre_wq = pre_emb = None
                if pre_br is not None:
                    c0 = CS[pre_br]
                    qdt0 = F32 if c0 <= 128 else F32R
                    pre_wq = pwq.tile([c0, c0], qdt0, tag="wq", name="pwq")
                    wsrc = wqt_d[pre_br][h, 0:c0, :]
                    esrc = embt_d[pre_br][0:c0, :]
                    if qdt0 is F32:
                        wsrc = wsrc.bitcast(F32)
                        esrc = esrc.bitcast(F32)
                    nc.sync.dma_start(pre_wq[:], wsrc)
                    pre_emb = pbig.tile([c0, N], qdt0, tag="big", name="pemb")
                    nc.sync.dma_start(pre_emb[:], esrc)

                # ---- K_h = emb_all @ Wk[h].T, layout [n, j]
                k_sb = [pk.tile([128, KV], F32R, tag="k", name=f"ksb{i}") for i in range(NT)]
                for jc in range(2):  # j chunks of 480
                    if jc == 0:
                        wk_ch = wk0
                    else:
                        wk_ch = pw.tile([KP, KT, 480], F32R, tag="w")
                        nc.sync.dma_start(
                            wk_ch[:],
                            wkt_d[h, :, :, ts(jc, 480)].rearrange("a b c -> b a c"),
                        )
                    for nt in range(NT):
                        pK = pmm.tile([128, 480], F32, tag="mm")
                        for kt in range(KT):
                            nc.tensor.matmul(
                                pK[:],
                                at_t[kt][:, ts(nt, 128)],
                                wk_ch[:, kt, :],
                                start=(kt == 0),
                                stop=(kt == KT - 1),
                            )
                        nc.scalar.copy(k_sb[nt][:, ts(jc, 480)], pK[:])

                # ---- VT_h = (emb_all @ Wv[h].T).T, layout [j, n], j-tiles of 120
                vt_sb = [pv.tile([JP, N], F32R, tag="v", name=f"vtsb{i}") for i in range(JT)]
                for half in range(2):
                    wv_ch = pw.tile([KP, KT, 480], F32R, tag="w")
                    nc.sync.dma_start(
                        wv_ch[:],
                        wvt_d[h, :, :, ts(half, 480)].rearrange("a b c -> b a c"),
                    )
                    for jx in range(4):
                        jt = half * 4 + jx
                        for nch in range(2):
                            pV = pmm.tile([JP, 512], F32, tag="mm")
                            for kt in range(KT):
                                nc.tensor.matmul(
                                    pV[:],
                                    wv_ch[:, kt, ts(jx, 120)],
                                    at_t[kt][:, ts(nch, 512)],
                                    start=(kt == 0),
                                    stop=(kt == KT - 1),
                                )
                            nc.vector.tensor_copy(vt_sb[jt][:, ts(nch, 512)], pV[:])

                for br in br_order:
                    c = CS[br]
                    nct = _nct(c)

                    # ---- weights + emb.T for this (head, branch)
                    qdt = F32 if c <= 128 else F32R  # fp32r is slow at free<256
                    if br == pre_br:
                        wq_t = [pre_wq]
                        emb_t = [pre_emb]
                    else:
                        wq_t = []
                        emb_t = []
                    for ckt in range(nct if br != pre_br else 0):
                        cc = min(128, c - ckt * 128)
                        wqt = pwq.tile([cc, c], qdt, tag="wq")
                        wsrc = wqt_d[br][h, ckt * 128 : ckt * 128 + cc, :]
                        esrc = embt_d[br][ckt * 128 : ckt * 128 + cc, :]
                        if qdt is F32:
                            wsrc = wsrc.bitcast(F32)
                            esrc = esrc.bitcast(F32)
                        nc.sync.dma_start(wqt[:], wsrc)
                        wq_t.append(wqt)
                        e = pbig.tile([cc, N], qdt, tag="big")
                        nc.sync.dma_start(e[:], esrc)
                        emb_t.append(e)

                    # ---- Q = emb @ WqT  [n, c]
                    q_sb = []
                    for nt in range(NT):
                        pQ = pmm.tile([128, c], F32, tag="mm")
                        for ckt in range(nct):
                            nc.tensor.matmul(
                                pQ[:],
                                emb_t[ckt][:, ts(nt, 128)],
                                wq_t[ckt][:],
                                start=(ckt == 0),
                                stop=(ckt == nct - 1),
                            )
                        q = pq.tile([128, c], F32R, tag="q")
                        nc.scalar.copy(q[:], pQ[:])
                        q_sb.append(q)

                    # ---- S = Q.T @ K  [c, 960]; evict + plane stats
                    # two 480-wide j-chunks, each its own 1-bank psum tile
                    stats = psm.tile([128, 4 * nct], F32, tag="st")
                    nc.vector.memset(stats[:], 0.0)
                    s_t = []
                    for ct in range(nct):
                        cc = min(128, c - ct * 128)
                        s = pbig.tile([cc, KV], F32R, tag="big")
                        for jc in range(2):
                            pS = pmm.tile([cc, 480], F32, tag="mm")
                            for nt in range(NT):
                                nc.tensor.matmul(
                                    pS[:],
                                    q_sb[nt][:, ct * 128 : ct * 128 + cc],
                                    k_sb[nt][:, ts(jc, 480)],
                                    start=(nt == 0),
                                    stop=(nt == NT - 1),
                                )
                            col = 2 * ct + jc
                            nc.vector.tensor_scalar(
                                s[:, ts(jc, 480)],
                                pS[:],
                                1.0,
                                None,
                                op0=ALU.mult,
                                op1=ALU.add,
                                accum_out=stats[0:cc, col : col + 1],
                            )
                            nc.scalar.activation(
                                pS[:],
                                pS[:],
                                AF.Square,
                                accum_out=stats[
                                    0:cc, 2 * nct + col : 2 * nct + col + 1
                                ],
                            )
                        s_t.append(s)

                    # ---- plane mean/var -> rsq, bias; broadcast to partitions
                    pst = ptp.tile([1, 4 * nct], F32, tag="tp")
                    nc.tensor.matmul(
                        pst[:],
                        inv_col[:, br : br + 1],
                        stats[:],
                        start=True,
                        stop=True,
                    )
                    ssq = psm.tile([1, 1], F32, tag="ssq")
                    nc.vector.reduce_sum(
                        out=ssq[:],
                        in_=pst[0:1, 2 * nct : 4 * nct],
                        axis=mybir.AxisListType.X,
                    )
                    # mu^2/var ~ 1e-5: drop the mean (softmax absorbs the shift)
                    lnv = psm.tile([1, 1], F32, tag="lnv")
                    nc.scalar.activation(lnv[:], ssq[:], AF.Ln, bias=eps_t[:])
                    rb = psm.tile([1, 1], F32, tag="rb")
                    nc.scalar.activation(rb[:], lnv[:], AF.Exp, scale=-0.5)
                    pbc = ptp.tile([128, 1], F32, tag="tp")
                    nc.tensor.matmul(pbc[:], ones_row[:], rb[:], start=True, stop=True)
                    rsqb = psm.tile([128, 1], F32, tag="rsqb")
                    nc.vector.tensor_copy(rsqb[:], pbc[:])

                    # ---- P = exp(rsq*S + bias) in place (f32r), rowsums
                    rec_t = []
                    for ct in range(nct):
                        cc = min(128, c - ct * 128)
                        rows = psm.tile([cc, 1], F32, tag="rows")
                        nc.scalar.activation(
                            s_t[ct][:],
                            s_t[ct][:].bitcast(F32),
                            AF.Exp,
                            scale=rsqb[0:cc, 0:1],
                            accum_out=rows[:],
                        )
                        rec = psm.tile([cc, 1], F32, tag="rec")
                        nc.vector.reciprocal(rec[:], rows[:])
                        rec_t.append(rec)

                    # ---- PT = P.T via PE transpose  [j, c]
                    ptile = ppt.tile([JP, JT, c], F32R, tag="pt")
                    for ct in range(nct):
                        cc = min(128, c - ct * 128)
                        for jt in range(JT):
                            tp_ps = ptp.tile([JP, cc], F32R, tag="tp")
                            nc.tensor.transpose(
                                tp_ps[:],
                                s_t[ct][:, ts(jt, JP)],
                                ident_r[0:cc, 0:cc],
                            )
                            nc.vector.tensor_copy(
                                ptile[:, jt, ct * 128 : ct * 128 + cc], tp_ps[:]
                            )

                    # ---- ctx = P @ V.T  [c, n], normalized per row; -> f32r
                    ctx_t = []
                    for ct in range(nct):
                        cc = min(128, c - ct * 128)
                        ctx = pctx.tile([cc, N], F32R, tag="ctx")
                        for nch in range(2):
                            pC = pmm.tile([cc, 512], F32, tag="mm")
                            for jt in range(JT):
                                nc.tensor.matmul(
                                    pC[:],
                                    ptile[:, jt, ct * 128 : ct * 128 + cc],
                                    vt_sb[jt][:, ts(nch, 512)],
                                    start=(jt == 0),
                                    stop=(jt == JT - 1),
                                )
                            nc.vector.tensor_scalar(
                                ctx[:, ts(nch, 512)],
                                pC[:],
                                rec_t[ct][:],
                                None,
                                op0=ALU.mult,
                            )
                        ctx_t.append(ctx)

                    # ---- out += ctx.T @ (Wo.T/4), accumulated in DRAM over heads
                    wo_t = []
                    for ckt in range(nct):
                        cc = min(128, c - ckt * 128)
                        wo = pwo.tile([cc, c], F32R, tag="wo")
                        nc.sync.dma_start(
                            wo[:], wot_d[br][ckt * 128 : ckt * 128 + cc, :]
                        )
                        wo_t.append(wo)
                    for cpt in range(nct):
                        ccp = min(128, c - cpt * 128)
                        ob = posb.tile([ccp, N], F32, tag="osb")
                        for nch in range(2):
                            pO = pmm.tile([ccp, 512], F32, tag="mm")
                            for ckt in range(nct):
                                nc.tensor.matmul(
                                    pO[:],
                                    wo_t[ckt][:, cpt * 128 : cpt * 128 + ccp],
                                    ctx_t[ckt][:, ts(nch, 512)],
                                    start=(ckt == 0),
                                    stop=(ckt == nct - 1),
                                )
                            nc.scalar.copy(ob[:, ts(nch, 512)], pO[:])
                        nc.gpsimd.dma_start(
                            o_d[br][cpt * 128 : cpt * 128 + ccp, :],
                            ob[:],
                            accum_op=ALU.add,
                        )

    nc.compile()
    return nc


_NC = None


def _get_nc():
    global _NC
    if _NC is None:
        _NC = build()
    return _NC


def _prep_core_inputs(inputs):
    """Host-side transposes/scaling -> per-core input maps."""
    f32 = np.float32
    emb = [np.asarray(inputs[f"emb{i + 1}"], dtype=f32) for i in range(4)]
    emb_all = np.asarray(inputs["emb_all"], dtype=f32)
    wq = [np.asarray(inputs[f"Wq{i + 1}"], dtype=f32) for i in range(4)]
    wk = np.asarray(inputs["Wk"], dtype=f32)
    wv = np.asarray(inputs["Wv"], dtype=f32)
    wo = [np.asarray(inputs[f"Wo{i + 1}"], dtype=f32) for i in range(4)]

    at = np.ascontiguousarray(emb_all.transpose(0, 2, 1)).reshape(B, KT, KP, N)
    embt = [np.ascontiguousarray(e.transpose(0, 2, 1)) for e in emb]
    wkt = np.ascontiguousarray(wk.transpose(0, 2, 1)).reshape(H, KT, KP, KV)
    wvt = np.ascontiguousarray(wv.transpose(0, 2, 1)).reshape(H, KT, KP, KV)
    scale = 1.0 / math.sqrt(KV)
    wqt = [np.ascontiguousarray(w.transpose(0, 2, 1)) * scale for w in wq]
    wot = [np.ascontiguousarray(w.T) * 0.25 for w in wo]

    in_maps = []
    for b in range(B):
        m = {"at": at[b], "wkt": wkt, "wvt": wvt}
        for i in range(4):
            m[f"embt{i + 1}"] = embt[i][b]
            m[f"wqt{i + 1}"] = wqt[i].astype(f32)
            m[f"wot{i + 1}"] = wot[i].astype(f32)
        in_maps.append(m)
    return in_maps


_LAST_EXEC_NS = None


def kernel(**inputs):
    global _LAST_EXEC_NS
    nc = _get_nc()
    in_maps = _prep_core_inputs(inputs)
    res = run_bass_kernel_spmd(nc, in_maps, list(range(NCORES)), trace=TRACE)
    _LAST_EXEC_NS = res.exec_time_ns
    outs = []
    for i, c in enumerate(CS):
        o = np.stack([res.results[b][f"o{i + 1}"] for b in range(B)], axis=0)
        outs.append(np.ascontiguousarray(o.transpose(0, 2, 1)).astype(np.float32))
    return tuple(outs)


# revision 23
# speedup vs baseline: 1.0092x; 1.0092x over previous
"""Multi-branch shared-KV cross-attention for Trainium2, batch-parallel on 8 cores.

Math (per batch b, per branch i with C in {64,128,256,512}):
  K   = emb_all @ Wk[h].T                    [N=1024, KV=960]  per head
  V   = emb_all @ Wv[h].T
  Q   = emb_i  @ (Wq[h]/sqrt(960)).T         [N, C]
  S   = Q.T @ K                              [C, KV]   (1/sqrt(960) folded in Wq)
  P   = softmax_j(rsq * S - rsq*mu),  rsq = 1/sqrt(var(S)+eps)  plane stats
  ctx = P @ V.T                              [C, N]
  out = sum_h (ctx_h.T @ (Wo.T/4))           [N, C]    (mean over 4 heads folded)

Each of the 8 NeuronCores processes one batch element; no collectives.
All heavy matmuls run in float32r (TF32-like, full PE rate at free-dim>=256,
~1.5e-4 l2 error/matmul); statistics, softmax and accumulation in fp32.
"""
import math
import sys

sys.path.insert(0, "/opt/trn_rl_repo")

import numpy as np

import concourse.bass as bass
import concourse.tile as tile
from concourse import bacc, mybir
from concourse.bass_utils import run_bass_kernel_spmd
from concourse.masks import make_identity

F32 = mybir.dt.float32
F32R = mybir.dt.float32r
AF = mybir.ActivationFunctionType
ALU = mybir.AluOpType
ts = bass.ts

B, N, H, KV = 8, 1024, 4, 960
CS = [64, 128, 256, 512]
EPS = 1e-5
NCORES = 8
KT, KP = 8, 120  # contraction tiles over KV: 960 = 8*120
NT = 8           # n tiles: 1024 = 8*128
JT, JP = 8, 120  # j tiles over KV for ctx: 960 = 8*120

TRACE = False  # test.py sets kernel.TRACE = True to profile


def _nct(c):
    return (c + 127) // 128


def _pin_act_tables():
    """Keep Exp/Ln/Square/Copy only in natural_log_exp_and_others so the
    table-load pass never alternates sets (each switch costs ~1.3us on the
    softmax critical path)."""
    from concourse import hw_specs

    if getattr(hw_specs, "_act_tables_pinned", False):
        return
    orig = hw_specs.get_activation_tables

    def patched(arch):
        t = orig(arch)
        mine = {AF.Exp, AF.Ln, AF.Square, AF.Copy, AF.Identity, AF.MemsetZero}
        for name in t:
            if name != "natural_log_exp_and_others":
                t[name] = t[name] - mine
        return t

    hw_specs.get_activation_tables = patched
    bacc.get_activation_tables = patched
    hw_specs._act_tables_pinned = True


def build():
    _pin_act_tables()
    nc = bacc.Bacc("TRN2", target_bir_lowering=False, debug=False, num_devices=NCORES)

    embt_d = [
        nc.dram_tensor(f"embt{i + 1}", [c, N], F32R, kind="ExternalInput").ap()
        for i, c in enumerate(CS)
    ]
    at_d = nc.dram_tensor("at", [KT, KP, N], F32R, kind="ExternalInput").ap()
    wkt_d = nc.dram_tensor("wkt", [H, KT, KP, KV], F32R, kind="ExternalInput").ap()
    wvt_d = nc.dram_tensor("wvt", [H, KT, KP, KV], F32R, kind="ExternalInput").ap()
    wqt_d = [
        nc.dram_tensor(f"wqt{i + 1}", [H, c, c], F32R, kind="ExternalInput").ap()
        for i, c in enumerate(CS)
    ]
    wot_d = [
        nc.dram_tensor(f"wot{i + 1}", [c, c], F32R, kind="ExternalInput").ap()
        for i, c in enumerate(CS)
    ]
    o_d = [
        nc.dram_tensor(f"o{i + 1}", [c, N], F32, kind="ExternalOutput").ap()
        for i, c in enumerate(CS)
    ]

    with tile.TileContext(nc) as tc:
        with (
            tc.tile_pool(name="big", bufs=9) as pbig,
            tc.tile_pool(name="w", bufs=2) as pw,
            tc.tile_pool(name="k", bufs=8) as pk,
            tc.tile_pool(name="v", bufs=8) as pv,
            tc.tile_pool(name="q", bufs=8) as pq,
            tc.tile_pool(name="wq", bufs=4) as pwq,
            tc.tile_pool(name="pt", bufs=1) as ppt,
            tc.tile_pool(name="ctx", bufs=4) as pctx,
            tc.tile_pool(name="wo", bufs=4) as pwo,
            tc.tile_pool(name="osb", bufs=2) as posb,
            tc.tile_pool(name="sm", bufs=4) as psm,
            tc.tile_pool(name="cst", bufs=1) as pcst,
            tc.tile_pool(name="mm", bufs=6, space="PSUM") as pmm,
            tc.tile_pool(name="tp", bufs=2, space="PSUM") as ptp,
        ):
            # ---- constants
            ident_f = pcst.tile([128, 128], F32, tag="identf")
            make_identity(nc, ident_f[:])
            ident_r = pcst.tile([128, 128], F32R, tag="identr")
            nc.vector.tensor_copy(ident_r[:], ident_f[:])
            inv_col = pcst.tile([128, 4], F32, tag="invc")
            for _br, _c in enumerate(CS):
                nc.vector.memset(inv_col[:, _br : _br + 1], 1.0 / (960.0 * _c))
            ones_row = pcst.tile([1, 128], F32, tag="onesr")
            nc.vector.memset(ones_row[:], 1.0)
            eps_t = pcst.tile([1, 1], F32, tag="eps")
            nc.vector.memset(eps_t[:], EPS)

            for h in range(H):
                # ---- first Wk chunk, then AT tiles (PE can start sooner)
                wk0 = pw.tile([KP, KT, 480], F32R, tag="w")
                for kt in range(KT):
                    nc.sync.dma_start(wk0[:, kt, :], wkt_d[h, kt, :, ts(0, 480)])
                at_t = []
                for kt in range(KT):
                    a = pbig.tile([KP, N], F32R, tag="big")
                    nc.sync.dma_start(a[:], at_d[kt])
                    at_t.append(a)

                # ---- K_h = emb_all @ Wk[h].T, layout [n, j]
                k_sb = [pk.tile([128, KV], F32R, tag="k", name=f"ksb{i}") for i in range(NT)]
                for jc in range(2):  # j chunks of 480
                    if jc == 0:
                        wk_ch = wk0
                    else:
                        wk_ch = pw.tile([KP, KT, 480], F32R, tag="w")
                        for kt in range(KT):
                            nc.sync.dma_start(
                                wk_ch[:, kt, :], wkt_d[h, kt, :, ts(jc, 480)]
                            )
                    for nt in range(NT):
                        pK = pmm.tile([128, 480], F32, tag="mm")
                        for kt in range(KT):
                            nc.tensor.matmul(
                                pK[:],
                                at_t[kt][:, ts(nt, 128)],
                                wk_ch[:, kt, :],
                                start=(kt == 0),
                                stop=(kt == KT - 1),
                            )
                        nc.scalar.copy(k_sb[nt][:, ts(jc, 480)], pK[:])

                # ---- VT_h = (emb_all @ Wv[h].T).T, layout [j, n], j-tiles of 120
                vt_sb = [pv.tile([JP, N], F32R, tag="v", name=f"vtsb{i}") for i in range(JT)]
                for half in range(2):
                    wv_ch = pw.tile([KP, KT, 480], F32R, tag="w")
                    for kt in range(KT):
                        nc.sync.dma_start(
                            wv_ch[:, kt, :], wvt_d[h, kt, :, ts(half, 480)]
                        )
                    for jx in range(4):
                        jt = half * 4 + jx
                        for nch in range(2):
                            pV = pmm.tile([JP, 512], F32, tag="mm")
                            for kt in range(KT):
                                nc.tensor.matmul(
                                    pV[:],
                                    wv_ch[:, kt, ts(jx, 120)],
                                    at_t[kt][:, ts(nch, 512)],
                                    start=(kt == 0),
                                    stop=(kt == KT - 1),
                                )
                            nc.vector.tensor_copy(vt_sb[jt][:, ts(nch, 512)], pV[:])

                br_order = (3, 2, 1, 0) if h == H - 1 else (0, 1, 2, 3)
                for br in br_order:
                    c = CS[br]
                    nct = _nct(c)

                    # ---- weights + emb.T for this (head, branch)
                    qdt = F32 if c <= 128 else F32R  # fp32r is slow at free<256
                    wq_t = []
                    emb_t = []
                    for ckt in range(nct):
                        cc = min(128, c - ckt * 128)
                        wqt = pwq.tile([cc, c], qdt, tag="wq")
                        wsrc = wqt_d[br][h, ckt * 128 : ckt * 128 + cc, :]
                        esrc = embt_d[br][ckt * 128 : ckt * 128 + cc, :]
                        if qdt is F32:
                            wsrc = wsrc.bitcast(F32)
                            esrc = esrc.bitcast(F32)
                        nc.sync.dma_start(wqt[:], wsrc)
                        wq_t.append(wqt)
                        e = pbig.tile([cc, N], qdt, tag="big")
                        nc.sync.dma_start(e[:], esrc)
                        emb_t.append(e)

                    # ---- Q = emb @ WqT  [n, c]
                    q_sb = []
                    for nt in range(NT):
                        pQ = pmm.tile([128, c], F32, tag="mm")
                        for ckt in range(nct):
                            nc.tensor.matmul(
                                pQ[:],
                                emb_t[ckt][:, ts(nt, 128)],
                                wq_t[ckt][:],
                                start=(ckt == 0),
                                stop=(ckt == nct - 1),
                            )
                        q = pq.tile([128, c], F32R, tag="q")
                        nc.scalar.copy(q[:], pQ[:])
                        q_sb.append(q)

                    # ---- S = Q.T @ K  [c, 960]; evict + plane stats
                    # two 480-wide j-chunks, each its own 1-bank psum tile
                    stats = psm.tile([128, 4 * nct], F32, tag="st")
                    nc.vector.memset(stats[:], 0.0)
                    s_t = []
                    for ct in range(nct):
                        cc = min(128, c - ct * 128)
                        s = pbig.tile([cc, KV], F32R, tag="big")
                        for jc in range(2):
                            pS = pmm.tile([cc, 480], F32, tag="mm")
                            for nt in range(NT):
                                nc.tensor.matmul(
                                    pS[:],
                                    q_sb[nt][:, ct * 128 : ct * 128 + cc],
                                    k_sb[nt][:, ts(jc, 480)],
                                    start=(nt == 0),
                                    stop=(nt == NT - 1),
                                )
                            col = 2 * ct + jc
                            nc.vector.tensor_scalar(
                                s[:, ts(jc, 480)],
                                pS[:],
                                1.0,
                                None,
                                op0=ALU.mult,
                                op1=ALU.add,
                                accum_out=stats[0:cc, col : col + 1],
                            )
                            nc.scalar.activation(
                                pS[:],
                                pS[:],
                                AF.Square,
                                accum_out=stats[
                                    0:cc, 2 * nct + col : 2 * nct + col + 1
                                ],
                            )
                        s_t.append(s)

                    # ---- plane mean/var -> rsq, bias; broadcast to partitions
                    pst = ptp.tile([1, 4 * nct], F32, tag="tp")
                    nc.tensor.matmul(
                        pst[:],
                        inv_col[:, br : br + 1],
                        stats[:],
                        start=True,
                        stop=True,
                    )
                    ssq = psm.tile([1, 1], F32, tag="ssq")
                    nc.vector.reduce_sum(
                        out=ssq[:],
                        in_=pst[0:1, 2 * nct : 4 * nct],
                        axis=mybir.AxisListType.X,
                    )
                    # mu^2/var ~ 1e-5: drop the mean (softmax absorbs the shift)
                    lnv = psm.tile([1, 1], F32, tag="lnv")
                    nc.scalar.activation(lnv[:], ssq[:], AF.Ln, bias=eps_t[:])
                    rb = psm.tile([1, 1], F32, tag="rb")
                    nc.scalar.activation(rb[:], lnv[:], AF.Exp, scale=-0.5)
                    pbc = ptp.tile([128, 1], F32, tag="tp")
                    nc.tensor.matmul(pbc[:], ones_row[:], rb[:], start=True, stop=True)
                    rsqb = psm.tile([128, 1], F32, tag="rsqb")
                    nc.vector.tensor_copy(rsqb[:], pbc[:])

                    # ---- P = exp(rsq*S + bias) in place (f32r), rowsums
                    rec_t = []
                    for ct in range(nct):
                        cc = min(128, c - ct * 128)
                        rows = psm.tile([cc, 1], F32, tag="rows")
                        nc.scalar.activation(
                            s_t[ct][:],
                            s_t[ct][:].bitcast(F32),
                            AF.Exp,
                            scale=rsqb[0:cc, 0:1],
                            accum_out=rows[:],
                        )
                        rec = psm.tile([cc, 1], F32, tag="rec")
                        nc.vector.reciprocal(rec[:], rows[:])
                        rec_t.append(rec)

                    # ---- PT = P.T via PE transpose  [j, c]
                    ptile = ppt.tile([JP, JT, c], F32R, tag="pt")
                    for ct in range(nct):
                        cc = min(128, c - ct * 128)
                        for jt in range(JT):
                            tp_ps = ptp.tile([JP, cc], F32R, tag="tp")
                            nc.tensor.transpose(
                                tp_ps[:],
                                s_t[ct][:, ts(jt, JP)],
                                ident_r[0:cc, 0:cc],
                            )
                            nc.vector.tensor_copy(
                                ptile[:, jt, ct * 128 : ct * 128 + cc], tp_ps[:]
                            )

                    # ---- ctx = P @ V.T  [c, n], normalized per row; -> f32r
                    ctx_t = []
                    for ct in range(nct):
                        cc = min(128, c - ct * 128)
                        ctx = pctx.tile([cc, N], F32R, tag="ctx")
                        for nch in range(2):
                            pC = pmm.tile([cc, 512], F32, tag="mm")
                            for jt in range(JT):
                                nc.tensor.matmul(
                                    pC[:],
                                    ptile[:, jt, ct * 128 : ct * 128 + cc],
                                    vt_sb[jt][:, ts(nch, 512)],
                                    start=(jt == 0),
                                    stop=(jt == JT - 1),
                                )
                            nc.vector.tensor_scalar(
                                ctx[:, ts(nch, 512)],
                                pC[:],
                                rec_t[ct][:],
                                None,
                                op0=ALU.mult,
                            )
                        ctx_t.append(ctx)

                    # ---- out += ctx.T @ (Wo.T/4), accumulated in DRAM over heads
                    wo_t = []
                    for ckt in range(nct):
                        cc = min(128, c - ckt * 128)
                        wo = pwo.tile([cc, c], F32R, tag="wo")
                        nc.sync.dma_start(
                            wo[:], wot_d[br][ckt * 128 : ckt * 128 + cc, :]
                        )
                        wo_t.append(wo)
                    for cpt in range(nct):
                        ccp = min(128, c - cpt * 128)
                        ob = posb.tile([ccp, N], F32, tag="osb")
                        for nch in range(2):
                            pO = pmm.tile([ccp, 512], F32, tag="mm")
                            for ckt in range(nct):
                                nc.tensor.matmul(
                                    pO[:],
                                    wo_t[ckt][:, cpt * 128 : cpt * 128 + ccp],
                                    ctx_t[ckt][:, ts(nch, 512)],
                                    start=(ckt == 0),
                                    stop=(ckt == nct - 1),
                                )
                            nc.scalar.copy(ob[:, ts(nch, 512)], pO[:])
                        nc.gpsimd.dma_start(
                            o_d[br][cpt * 128 : cpt * 128 + ccp, :],
                            ob[:],
                            accum_op=ALU.add,
                        )

    nc.compile()
    return nc


_NC = None


def _get_nc():
    global _NC
    if _NC is None:
        _NC = build()
    return _NC


def _prep_core_inputs(inputs):
    """Host-side transposes/scaling -> per-core input maps."""
    f32 = np.float32
    emb = [np.asarray(inputs[f"emb{i + 1}"], dtype=f32) for i in range(4)]
    emb_all = np.asarray(inputs["emb_all"], dtype=f32)
    wq = [np.asarray(inputs[f"Wq{i + 1}"], dtype=f32) for i in range(4)]
    wk = np.asarray(inputs["Wk"], dtype=f32)
    wv = np.asarray(inputs["Wv"], dtype=f32)
    wo = [np.asarray(inputs[f"Wo{i + 1}"], dtype=f32) for i in range(4)]

    at = np.ascontiguousarray(emb_all.transpose(0, 2, 1)).reshape(B, KT, KP, N)
    embt = [np.ascontiguousarray(e.transpose(0, 2, 1)) for e in emb]
    wkt = np.ascontiguousarray(wk.transpose(0, 2, 1)).reshape(H, KT, KP, KV)
    wvt = np.ascontiguousarray(wv.transpose(0, 2, 1)).reshape(H, KT, KP, KV)
    scale = 1.0 / math.sqrt(KV)
    wqt = [np.ascontiguousarray(w.transpose(0, 2, 1)) * scale for w in wq]
    wot = [np.ascontiguousarray(w.T) * 0.25 for w in wo]

    in_maps = []
    for b in range(B):
        m = {"at": at[b], "wkt": wkt, "wvt": wvt}
        for i in range(4):
            m[f"embt{i + 1}"] = embt[i][b]
            m[f"wqt{i + 1}"] = wqt[i].astype(f32)
            m[f"wot{i + 1}"] = wot[i].astype(f32)
        in_maps.append(m)
    return in_maps


_LAST_EXEC_NS = None


def kernel(**inputs):
    global _LAST_EXEC_NS
    nc = _get_nc()
    in_maps = _prep_core_inputs(inputs)
    res = run_bass_kernel_spmd(nc, in_maps, list(range(NCORES)), trace=TRACE)
    _LAST_EXEC_NS = res.exec_time_ns
    outs = []
    for i, c in enumerate(CS):
        o = np.stack([res.results[b][f"o{i + 1}"] for b in range(B)], axis=0)
        outs.append(np.ascontiguousarray(o.transpose(0, 2, 1)).astype(np.float32))
    return tuple(outs)
